# revision 18
# baseline (speedup 1.0000x reference)
"""Causal multi-head attention (B=4, S=2048, D=1024, H=16, dk=64) on 8 TRN2
NeuronCores.

Sharding: core c = (batch b = c // 2, head-group g = c % 2 of 8 heads).
Each core computes, for its batch and its 8 heads:
    Q.T, K.T (feature-major) and V (seq-major) projections,
    S.T = K_h @ Q_h.T tiles (keys on partitions, queries on free dim),
    causal mask (additive -1e30 on the 128-wide diagonal band),
    exp (scale 1/sqrt(dk) folded into the ACT activation),
    A.T = [V_h | ones].T @ expS.T accumulated in PSUM (row 64 = softmax
    denominator, obtained for free), normalization via DVE reciprocal +
    a K=1 ones-matmul partition broadcast,
    partial out.T = W_o_slice.T-chunks @ A.T  (summed on host across the
    2 head-group cores of each batch).

All matmuls run as float32r (full PE speed for free dim >= 256; inputs are
fp32 bit patterns truncated to FP22 on read).
"""

import numpy as np

import concourse.bacc as bacc
import concourse.tile as tile
from concourse import mybir
from concourse import bass_utils

F32 = mybir.dt.float32
F32R = mybir.dt.float32r
BF16 = mybir.dt.bfloat16
P = 128          # partitions
S = 2048         # sequence length
D = 1024         # model dim
FLOC = 512       # local features per core (8 heads x 64)
HLOC = 8         # heads per core
DK = 64
NB = 4           # seq blocks of 512 (query blocks)
KC = 16          # key chunks of 128
DC = 8           # D chunks of 128
FC = 4           # local-feature chunks of 128 (= head pairs)
OB = 8           # output-D blocks of 128
NEG = -1.0e30
SCALE = 0.125    # 1 / sqrt(dk)

_CACHE = {}


def _emit(nc):
    xT = nc.dram_tensor("xT", [D, S], BF16, kind="ExternalInput")
    wqT = nc.dram_tensor("wqT", [D, FLOC], BF16, kind="ExternalInput")
    wkT = nc.dram_tensor("wkT", [D, FLOC], BF16, kind="ExternalInput")
    wvT = nc.dram_tensor("wvT", [D, FLOC], BF16, kind="ExternalInput")
    woT = nc.dram_tensor("woT", [FLOC, D], BF16, kind="ExternalInput")
    onesc = nc.dram_tensor("onesc", [P, P], BF16, kind="ExternalInput")
    maskb = nc.dram_tensor("maskb", [P, 2 * P], F32, kind="ExternalInput")
    outT = nc.dram_tensor("outT", [D, S], BF16, kind="ExternalOutput")

    # DRAM APs in on-chip layouts
    xT_a = xT.ap().rearrange("(c p) s -> p c s", p=P)      # [128, 8, 2048]
    wqT_a = wqT.ap().rearrange("(c p) f -> p c f", p=P)    # [128, 8, 512]
    wkT_a = wkT.ap().rearrange("(c p) f -> p c f", p=P)
    wvT_a = wvT.ap().rearrange("(c p) f -> p c f", p=P)
    woT_a = woT.ap().rearrange("(c p) j -> p c j", p=P)    # [128, 4, 1024]
    outT_a = outT.ap().rearrange("(c p) s -> p c s", p=P)                # [128, 8, 2048]

    with tile.TileContext(nc) as tc:
        import contextlib
        ctx = contextlib.ExitStack()
        with ctx:
            persist = ctx.enter_context(tc.tile_pool(name="persist", bufs=1))
            wpool = ctx.enter_context(tc.tile_pool(name="w", bufs=1))
            xtp = ctx.enter_context(tc.tile_pool(name="xt", bufs=2))
            qtp = ctx.enter_context(tc.tile_pool(name="qt", bufs=2))
            ep = ctx.enter_context(tc.tile_pool(name="e", bufs=18))
            atp = ctx.enter_context(tc.tile_pool(name="at", bufs=1))
            repp = ctx.enter_context(tc.tile_pool(name="rep", bufs=2))
            outp = ctx.enter_context(tc.tile_pool(name="out", bufs=4))
            invp = ctx.enter_context(tc.tile_pool(name="inv", bufs=2))
            pss = ctx.enter_context(tc.tile_pool(name="pss", bufs=2, space="PSUM"))
            pss2 = ctx.enter_context(tc.tile_pool(name="pss2", bufs=2, space="PSUM"))
            pav = ctx.enter_context(tc.tile_pool(name="pav", bufs=2, space="PSUM"))

            # static tiles; wq is split per D-chunk into separate tiles so
            # the first projection matmuls only wait on the pieces they read
            # (Tile tracks dependencies at whole-tile granularity).
            wq_sb = [wpool.tile([P, FLOC], BF16, tag=f"wq{dc}", name=f"wq{dc}")
                     for dc in range(DC)]
            wk_sb = [wpool.tile([P, FLOC], BF16, tag=f"wk{dc}", name=f"wk{dc}")
                     for dc in range(DC)]
            wv_sb = [wpool.tile([P, FLOC], BF16, tag=f"wv{dc}", name=f"wv{dc}")
                     for dc in range(DC)]
            wo_sb = persist.tile([P, FC, D], BF16, tag="wo")
            mask_sb = persist.tile([P, 2 * P], F32, tag="mask")

            kt_blocks = []
            v_blocks = []
            qt_blocks = []

            def phase1_chunks(sb):
                """QKV projections for seq block sb, as a list of emission
                chunks so they can be interleaved with attention work."""
                chunks = []

                state = {}

                def setup():
                    xt_t = [xtp.tile([P, 512], BF16, tag=f"xt{dc}", name=f"xt{dc}")
                            for dc in range(DC)]
                    qs = [nc.sync, nc.scalar]
                    for dc in range(DC):
                        if sb == 0:
                            # interleave the wq pieces so the first projection
                            # group streams right behind the DMA dispatches;
                            # round-robin engine DGE queues to parallelize
                            # the startup stream on hardware
                            qs[dc % 2].dma_start(wq_sb[dc][:], wqT_a[:, dc, :])
                            qs[(dc + 1) % 2].dma_start(
                                xt_t[dc][:],
                                xT_a[:, dc, sb * 512:(sb + 1) * 512])
                        else:
                            nc.sync.dma_start(
                                xt_t[dc][:],
                                xT_a[:, dc, sb * 512:(sb + 1) * 512])
                    qt_t = qtp.tile([P, FC, 512], BF16, tag="qt")
                    kt_t = persist.tile([P, FC, 512], BF16, tag=f"kt{sb}")
                    v_t = persist.tile([P, 4, HLOC, DK + 1], BF16, tag=f"v{sb}")
                    qt_blocks.append(qt_t)
                    kt_blocks.append(kt_t)
                    v_blocks.append(v_t)
                    # ones column of V_aug (softmax denominators)
                    nc.sync.dma_start(
                        v_t[:, :, :, DK],
                        onesc.ap()[:, 0:32]
                        .rearrange("p (a b) -> p a b", a=4),
                    )
                    state.update(xt=xt_t, qt=qt_t, kt=kt_t, v=v_t)

                chunks.append(setup)

                def q_chunk(fc):
                    def run():
                        ps_q = pss.tile([P, 512], F32, tag="s")
                        for dc in range(DC):
                            nc.tensor.matmul(
                                ps_q[:], wq_sb[dc][:, fc * P:(fc + 1) * P],
                                state["xt"][dc][:],
                                start=(dc == 0), stop=(dc == DC - 1),
                            )
                        nc.scalar.copy(state["qt"][:, fc, :], ps_q[:])
                    return run

                def k_chunk(fc):
                    def run():
                        ps_k = pss.tile([P, 512], F32, tag="s")
                        for dc in range(DC):
                            nc.tensor.matmul(
                                ps_k[:], wk_sb[dc][:, fc * P:(fc + 1) * P],
                                state["xt"][dc][:],
                                start=(dc == 0), stop=(dc == DC - 1),
                            )
                        nc.scalar.copy(state["kt"][:, fc, :], ps_k[:])
                    return run

                def v_chunk(sc):
                    def run():
                        ps_v = pss.tile([P, 512], F32, tag="s")
                        for dc in range(DC):
                            nc.tensor.matmul(
                                ps_v[:], state["xt"][dc][:, sc * P:(sc + 1) * P],
                                wv_sb[dc][:],
                                start=(dc == 0), stop=(dc == DC - 1),
                            )
                        nc.scalar.copy(
                            state["v"][:, sc, :, 0:DK],
                            ps_v[:].rearrange("p (h d) -> p h d", h=HLOC),
                        )
                    return run

                for fc in range(FC):
                    chunks.append(q_chunk(fc))
                for fc in range(FC):
                    chunks.append(k_chunk(fc))
                for sc in range(4):
                    chunks.append(v_chunk(sc))
                return chunks

            def phase1(sb):
                for ch in phase1_chunks(sb):
                    ch()

            def attention_chunks(qb):
                """Attention + output projection for query block qb.

                Returns (pairs, outproj) where pairs[p] = (scores_part,
                av_part): scores_part emits scores+mask+exp (and any
                pend-overflow AV), av_part flushes the remaining AV matmuls
                and normalizes. Emitting them as separate chunks lets the
                qb=0 schedule start scores before the V projections exist.
                """
                qt_t = qt_blocks[qb]
                at_t = [atp.tile([P, 512], BF16, tag=f"at{qb}_{p}",
                                 name=f"at{qb}_{p}") for p in range(FC)]
                last = 4 * qb + 3

                def make_pair(p):  # head pair
                    st = {}

                    def emit_av(e_pair, col0, kc):
                        v_t = v_blocks[kc // 4]
                        nc.tensor.matmul(
                            st["a0"][:, col0:], v_t[:, kc % 4, 2 * p, :],
                            e_pair[:, 0, col0:], start=(kc == 0), stop=(kc == last),
                        )
                        nc.tensor.matmul(
                            st["a1"][:, col0:], v_t[:, kc % 4, 2 * p + 1, :],
                            e_pair[:, 1, col0:], start=(kc == 0), stop=(kc == last),
                        )

                    def scores_part():
                        st["a0"] = pav.tile([DK + 1, 512], F32, tag="av", name="ps_a0")
                        st["a1"] = pav.tile([DK + 1, 512], F32, tag="av", name="ps_a1")
                        pend = st["pend"] = []
                        for kc in range(last + 1):
                            r = kc - 4 * qb
                            col0 = P * r if r >= 0 else 0
                            mcol0 = col0
                            kt_t = kt_blocks[kc // 4]
                            ks = slice((kc % 4) * P, (kc % 4 + 1) * P)
                            ps_s = pss2.tile([P, 2, 512], F32, tag="s2")
                            nc.tensor.matmul(
                                ps_s[:, 0, mcol0:], kt_t[0:DK, p, ks],
                                qt_t[0:DK, p, mcol0:], start=True, stop=True,
                            )
                            nc.tensor.matmul(
                                ps_s[:, 1, mcol0:], kt_t[DK:P, p, ks],
                                qt_t[DK:P, p, mcol0:], start=True, stop=True,
                            )
                            if r >= 0:
                                mstart = mcol0 if r == 3 else col0
                                region = slice(mstart, col0 + P)
                                w = col0 + P - mstart
                                nc.vector.tensor_tensor(
                                    ps_s[:, :, region],
                                    ps_s[:, :, region],
                                    mask_sb[:, 2 * P - w:]
                                    .unsqueeze(1).to_broadcast([P, 2, w]),
                                    mybir.AluOpType.add,
                                )
                            e_pair = ep.tile([P, 2, 512], BF16, tag="e")
                            nc.scalar.activation(
                                e_pair[:, :, mcol0:], ps_s[:, :, mcol0:],
                                mybir.ActivationFunctionType.Exp, scale=SCALE,
                            )
                            pend.append((e_pair, mcol0, kc))
                            if len(pend) > 5:
                                emit_av(*pend.pop(0))

                    def av_part():
                        for it in st["pend"]:
                            emit_av(*it)
                        # normalize: at[f, q] = a[f, q] / a[64, q].
                        # The 1/denom row broadcast runs on GPSIMD (idle
                        # engine) instead of a PE ones-matmul.
                        for half, ps_a in ((0, st["a0"]), (1, st["a1"])):
                            inv_r = invp.tile([1, 512], F32, tag="invr")
                            with nc.allow_low_precision(reason="softmax recip"):
                                nc.vector.reciprocal(inv_r[:], ps_a[DK:DK + 1, :])
                            rep = repp.tile([DK, 512], F32, tag="rep")
                            nc.gpsimd.partition_broadcast(rep[:], inv_r[:])
                            nc.vector.tensor_tensor(
                                at_t[p][half * DK:(half + 1) * DK, :],
                                ps_a[0:DK, :], rep[:], mybir.AluOpType.mult,
                            )

                    return scores_part, av_part
                def outproj():
                    # output projection: outT[j, q] partial
                    for ob in range(OB):
                        ps_o = pss.tile([P, 512], F32, tag="s")
                        for fc in range(FC):
                            nc.tensor.matmul(
                                ps_o[:], wo_sb[:, fc, ob * P:(ob + 1) * P],
                                at_t[fc][:],
                                start=(fc == 0), stop=(fc == FC - 1),
                            )
                        o_t = outp.tile([P, 512], BF16, tag="o")
                        nc.vector.tensor_copy(o_t[:], ps_o[:])
                        nc.sync.dma_start(
                            outT_a[:, ob, qb * 512:(qb + 1) * 512], o_t[:])

                return [make_pair(p) for p in range(FC)], outproj

            # Emission. qb=0: weave the attention pairs directly into
            # phase1(0) — pair p's scores only need the fc=p chunks of
            # qt/kt, so they run as soon as (Qp, Kp) land instead of after
            # the whole projection phase. Weight streams ride the Pool
            # SWDGE queue, parallel to the wq/xt HWDGE stream, ordered to
            # match first use (wk before wv before wo). qb>=1: attention
            # (qb) interleaved with phase1(qb+1) — one block ahead of the
            # attention that consumes it.
            ph = phase1_chunks(0)       # [setup, Q0..Q3, K0..K3, V0..V3]
            ph[0]()                      # wq + xt(0) DMAs
            nc.sync.dma_start(mask_sb[:], maskb.ap())
            for dc in range(DC):
                nc.gpsimd.dma_start(wk_sb[dc][:], wkT_a[:, dc, :])
            for dc in range(DC):
                nc.gpsimd.dma_start(wv_sb[dc][:], wvT_a[:, dc, :])
            nc.gpsimd.dma_start(wo_sb[:], woT_a)
            att0, opj0 = attention_chunks(0)
            ph[1]()                      # Q0
            ph[2]()                      # Q1
            ph[5]()                      # K0
            att0[0][0]()                 # scores(p0)
            ph[3]()                      # Q2
            ph[6]()                      # K1
            att0[1][0]()                 # scores(p1)
            ph[4]()                      # Q3
            ph[7]()                      # K2
            att0[2][0]()                 # scores(p2)
            ph[8]()                      # K3
            att0[3][0]()                 # scores(p3)
            nxt = phase1_chunks(1)
            nxt[0]()                     # xt(1) DMA prefetch (xt double-buffered)
            ph[9]()                      # V0
            ph[10]()                     # V1
            ph[11]()                     # V2
            ph[12]()                     # V3
            att0[0][1]()                 # av(p0)
            nxt[1]()                     # Q0(1) — covers the norm latency
            att0[1][1]()                 # av(p1)
            nxt[2]()                     # Q1(1)
            att0[2][1]()                 # av(p2)
            nxt[3]()                     # Q2(1)
            att0[3][1]()                 # av(p3)
            nxt[4]()                     # Q3(1)
            opjs = [opj0]
            for ch in nxt[5:]:           # K(1), V(1)
                ch()
            for qb in range(1, NB - 1):
                pairs, opj = attention_chunks(qb)
                att = [c for pr in pairs for c in pr]
                opjs.append(opj)
                nxt = phase1_chunks(qb + 1)
                seq = []
                seq.append(nxt[0])       # xt DMA prefetch first
                k = 1
                for i, pc in enumerate(att):
                    seq.append(pc)
                    if i % 2 == 0:       # pace projections behind scores
                        for _ in range(2):
                            if k < len(nxt):
                                seq.append(nxt[k])
                                k += 1
                while k < len(nxt):
                    seq.append(nxt[k])
                    k += 1
                for ch in seq:
                    ch()
            # Last block: the deferred output projections are the only
            # remaining PE-dense work — weave them between the AV parts so
            # the exp backlog on ACT is hidden behind them.
            pairs, opj3 = attention_chunks(NB - 1)
            sc, av = zip(*pairs)
            sc[0](); av[0]()
            sc[1](); opjs[0](); av[1]()
            sc[2](); opjs[1](); av[2]()
            sc[3](); opjs[2](); av[3]()
            opj3()


def _build():
    nc = bacc.Bacc("TRN2", target_bir_lowering=False, debug=False)
    _emit(nc)
    nc.compile()
    return nc


def _make_in_maps(x, W_q, W_k, W_v, W_o):
    import ml_dtypes
    bf = ml_dtypes.bfloat16
    onesc = np.ones((P, P), dtype=bf)
    kk = np.arange(P)[:, None]
    jj = np.arange(P)[None, :]
    band = np.where(kk <= jj, 0.0, NEG).astype(np.float32)
    # [128, 256]: first 128 cols fully masked (r=3 widened tiles), then the
    # triangular diagonal band
    maskb = np.concatenate(
        [np.full((P, P), NEG, np.float32), band], axis=1)
    in_maps = []
    for c in range(8):
        b, g = divmod(c, 2)
        cols = slice(g * FLOC, (g + 1) * FLOC)
        in_maps.append({
            "xT": np.ascontiguousarray(x[b].T).astype(bf),
            "wqT": np.ascontiguousarray(W_q[cols, :].T).astype(bf),
            "wkT": np.ascontiguousarray(W_k[cols, :].T).astype(bf),
            "wvT": np.ascontiguousarray(W_v[cols, :].T).astype(bf),
            "woT": np.ascontiguousarray(W_o[:, cols].T).astype(bf),
            "onesc": onesc,
            "maskb": maskb,
        })
    return in_maps


def kernel(x, W_q, W_k, W_v, W_o):
    x = np.asarray(x, dtype=np.float32)
    W_q = np.asarray(W_q, dtype=np.float32)
    W_k = np.asarray(W_k, dtype=np.float32)
    W_v = np.asarray(W_v, dtype=np.float32)
    W_o = np.asarray(W_o, dtype=np.float32)
    if "nc" not in _CACHE:
        _CACHE["nc"] = _build()
    nc = _CACHE["nc"]
    in_maps = _make_in_maps(x, W_q, W_k, W_v, W_o)
    res = bass_utils.run_bass_kernel_spmd(nc, in_maps, core_ids=list(range(8)))
    B = x.shape[0]
    out = np.empty((B, S, D), dtype=np.float32)
    for b in range(B):
        acc = (res.results[2 * b]["outT"].astype(np.float32)
               + res.results[2 * b + 1]["outT"].astype(np.float32))
        out[b] = acc.T
    return out



# revision 38
# speedup vs baseline: 2.2437x; 2.2437x over previous
"""Causal multi-head attention (B=4, S=2048, D=1024, H=16, dk=64) on 8 TRN2
NeuronCores.

Sharding: core c = (batch b = c // 2, head-group g = c % 2 of 8 heads).
Each core computes, for its batch and its 8 heads:
    Q.T, K.T (feature-major) and V (seq-major) projections,
    S.T = K_h @ Q_h.T tiles (keys on partitions, queries on free dim; the
    two heads of a pair land in disjoint PE row groups and run row-tiled),
    causal mask (additive -1e30 on the 128-wide diagonal band),
    exp (scale 1/sqrt(dk) folded into the ACT activation),
    A.T = [V_h | ones].T @ expS.T accumulated in PSUM (row 64 = softmax
    denominator, obtained for free), normalization via DVE reciprocal +
    GPSIMD partition_broadcast (keeps the PE free of broadcast matmuls),
    partial out.T = W_o_slice.T-chunks @ A.T  (summed on host across the
    2 head-group cores of each batch).

All matmul operands are bf16 (fp32 PSUM accumulation; max rel err vs the
fp32 reference ~5e-3, well under the 2e-2 gate). Inputs are converted to
bf16 on the host, halving DMA traffic; output partials return as bf16 and
are summed in fp32 on the host.

Schedule: phase1(qb) projections run one query-block ahead, woven into
attention(qb-1). For qb=0 the attention pairs are woven directly into
phase1(0) per (Qp, Kp) chunk. Weight streams ride the Pool SWDGE queue in
parallel with the wq/xt HWDGE stream. All four output projections are
deferred to the final attention block, where they are the PE-dense filler
that hides the ACT exp backlog (every attention block standalone is
ACT-bound; the projections are what balance it).
"""

import numpy as np

import concourse.bacc as bacc
import concourse.tile as tile
from concourse import mybir
from concourse import bass_utils

F32 = mybir.dt.float32
F32R = mybir.dt.float32r
BF16 = mybir.dt.bfloat16
P = 128          # partitions
S = 2048         # sequence length
D = 1024         # model dim
FLOC = 512       # local features per core (8 heads x 64)
HLOC = 8         # heads per core
DK = 64
NB = 4           # seq blocks of 512 (query blocks)
KC = 16          # key chunks of 128
DC = 8           # D chunks of 128
FC = 4           # local-feature chunks of 128 (= head pairs)
OB = 8           # output-D blocks of 128
NEG = -1.0e30
SCALE = 0.125    # 1 / sqrt(dk)

_CACHE = {}


def _emit(nc):
    xT = nc.dram_tensor("xT", [D, S], BF16, kind="ExternalInput")
    wqT = nc.dram_tensor("wqT", [D, FLOC], BF16, kind="ExternalInput")
    wkT = nc.dram_tensor("wkT", [D, FLOC], BF16, kind="ExternalInput")
    wvT = nc.dram_tensor("wvT", [D, FLOC], BF16, kind="ExternalInput")
    woT = nc.dram_tensor("woT", [FLOC, D], BF16, kind="ExternalInput")
    maskb = nc.dram_tensor("maskb", [P, 2 * P], F32, kind="ExternalInput")
    outT = nc.dram_tensor("outT", [D, S], BF16, kind="ExternalOutput")

    # DRAM APs in on-chip layouts
    xT_a = xT.ap().rearrange("(c p) s -> p c s", p=P)      # [128, 8, 2048]
    wqT_a = wqT.ap().rearrange("(c p) f -> p c f", p=P)    # [128, 8, 512]
    wkT_a = wkT.ap().rearrange("(c p) f -> p c f", p=P)
    wvT_a = wvT.ap().rearrange("(c p) f -> p c f", p=P)
    woT_a = woT.ap().rearrange("(c p) j -> p c j", p=P)    # [128, 4, 1024]
    outT_a = outT.ap().rearrange("(c p) s -> p c s", p=P)                # [128, 8, 2048]

    with tile.TileContext(nc) as tc:
        import contextlib
        ctx = contextlib.ExitStack()
        with ctx:
            persist = ctx.enter_context(tc.tile_pool(name="persist", bufs=1))
            wpool = ctx.enter_context(tc.tile_pool(name="w", bufs=1))
            xtp = ctx.enter_context(tc.tile_pool(name="xt", bufs=2))
            qtp = ctx.enter_context(tc.tile_pool(name="qt", bufs=2))
            ep = ctx.enter_context(tc.tile_pool(name="e", bufs=18))
            atp = ctx.enter_context(tc.tile_pool(name="at", bufs=1))
            repp = ctx.enter_context(tc.tile_pool(name="rep", bufs=4))
            outp = ctx.enter_context(tc.tile_pool(name="out", bufs=4))
            invp = ctx.enter_context(tc.tile_pool(name="inv", bufs=4))
            pss = ctx.enter_context(tc.tile_pool(name="pss", bufs=2, space="PSUM"))
            pss2 = ctx.enter_context(tc.tile_pool(name="pss2", bufs=2, space="PSUM"))
            pav = ctx.enter_context(tc.tile_pool(name="pav", bufs=2, space="PSUM"))

            # static tiles; wq is split per D-chunk into separate tiles so
            # the first projection matmuls only wait on the pieces they read
            # (Tile tracks dependencies at whole-tile granularity).
            wq_sb = [wpool.tile([P, FLOC], BF16, tag=f"wq{dc}", name=f"wq{dc}")
                     for dc in range(DC)]
            wk_sb = [wpool.tile([P, FLOC], BF16, tag=f"wk{dc}", name=f"wk{dc}")
                     for dc in range(DC)]
            wv_sb = [wpool.tile([P, FLOC], BF16, tag=f"wv{dc}", name=f"wv{dc}")
                     for dc in range(DC)]
            wo_sb = persist.tile([P, FC, D], BF16, tag="wo")
            mask_sb = persist.tile([P, 2 * P], F32, tag="mask")

            kt_blocks = []
            v_blocks = []
            qt_blocks = []

            def phase1_chunks(sb):
                """QKV projections for seq block sb, as a list of emission
                chunks so they can be interleaved with attention work."""
                chunks = []

                state = {}

                def setup():
                    xt_t = [xtp.tile([P, 512], BF16, tag=f"xt{dc}", name=f"xt{dc}")
                            for dc in range(DC)]
                    qs = [nc.sync, nc.scalar]
                    for dc in range(DC):
                        if sb == 0:
                            # interleave the wq pieces so the first projection
                            # group streams right behind the DMA dispatches;
                            # two engine DGE queues in parallel
                            qs[dc % 2].dma_start(wq_sb[dc][:], wqT_a[:, dc, :])
                            qs[(dc + 1) % 2].dma_start(
                                xt_t[dc][:],
                                xT_a[:, dc, sb * 512:(sb + 1) * 512])
                        else:
                            nc.sync.dma_start(
                                xt_t[dc][:],
                                xT_a[:, dc, sb * 512:(sb + 1) * 512])
                    qt_t = qtp.tile([P, FC, 512], BF16, tag="qt")
                    kt_t = persist.tile([P, FC, 512], BF16, tag=f"kt{sb}")
                    v_t = persist.tile([P, 4, HLOC, DK + 1], BF16, tag=f"v{sb}")
                    qt_blocks.append(qt_t)
                    kt_blocks.append(kt_t)
                    v_blocks.append(v_t)
                    # ones column of V_aug (softmax denominators)
                    nc.gpsimd.memset(v_t[:, :, :, DK], 1.0)
                    state.update(xt=xt_t, qt=qt_t, kt=kt_t, v=v_t)

                chunks.append(setup)

                def q_chunk(fc):
                    def run():
                        ps_q = pss.tile([P, 512], F32, tag="s")
                        for dc in range(DC):
                            nc.tensor.matmul(
                                ps_q[:], wq_sb[dc][:, fc * P:(fc + 1) * P],
                                state["xt"][dc][:],
                                start=(dc == 0), stop=(dc == DC - 1),
                            )
                        nc.scalar.copy(state["qt"][:, fc, :], ps_q[:])
                    return run

                def k_chunk(fc):
                    def run():
                        ps_k = pss.tile([P, 512], F32, tag="s")
                        for dc in range(DC):
                            nc.tensor.matmul(
                                ps_k[:], wk_sb[dc][:, fc * P:(fc + 1) * P],
                                state["xt"][dc][:],
                                start=(dc == 0), stop=(dc == DC - 1),
                            )
                        nc.scalar.copy(state["kt"][:, fc, :], ps_k[:])
                    return run

                def v_chunk(sc):
                    def run():
                        ps_v = pss.tile([P, 512], F32, tag="s")
                        for dc in range(DC):
                            nc.tensor.matmul(
                                ps_v[:], state["xt"][dc][:, sc * P:(sc + 1) * P],
                                wv_sb[dc][:],
                                start=(dc == 0), stop=(dc == DC - 1),
                            )
                        nc.scalar.copy(
                            state["v"][:, sc, :, 0:DK],
                            ps_v[:].rearrange("p (h d) -> p h d", h=HLOC),
                        )
                    return run

                for fc in range(FC):
                    chunks.append(q_chunk(fc))
                for fc in range(FC):
                    chunks.append(k_chunk(fc))
                for sc in range(4):
                    chunks.append(v_chunk(sc))
                return chunks

            def phase1(sb):
                for ch in phase1_chunks(sb):
                    ch()

            def attention_chunks(qb):
                """Attention + output projection for query block qb.

                Returns (pairs, outproj) where pairs[p] = (scores_part,
                av_part): scores_part emits scores+mask+exp (and any
                pend-overflow AV), av_part flushes the remaining AV matmuls
                and normalizes. Emitting them as separate chunks lets the
                qb=0 schedule start scores before the V projections exist.
                """
                qt_t = qt_blocks[qb]
                at_t = [atp.tile([P, 512], BF16, tag=f"at{qb}_{p}",
                                 name=f"at{qb}_{p}") for p in range(FC)]
                last = 4 * qb + 3

                def make_pair(p):  # head pair
                    st = {}

                    def emit_av(e_pair, col0, kc):
                        v_t = v_blocks[kc // 4]
                        nc.tensor.matmul(
                            st["a0"][:, col0:], v_t[:, kc % 4, 2 * p, :],
                            e_pair[:, 0, col0:], start=(kc == 0), stop=(kc == last),
                        )
                        nc.tensor.matmul(
                            st["a1"][:, col0:], v_t[:, kc % 4, 2 * p + 1, :],
                            e_pair[:, 1, col0:], start=(kc == 0), stop=(kc == last),
                        )

                    def scores_part():
                        st["a0"] = pav.tile([DK + 1, 512], F32, tag="av", name="ps_a0")
                        st["a1"] = pav.tile([DK + 1, 512], F32, tag="av", name="ps_a1")
                        pend = st["pend"] = []
                        for kc in range(last + 1):
                            r = kc - 4 * qb
                            col0 = P * r if r >= 0 else 0
                            mcol0 = col0
                            kt_t = kt_blocks[kc // 4]
                            ks = slice((kc % 4) * P, (kc % 4 + 1) * P)
                            ps_s = pss2.tile([P, 2, 512], F32, tag="s2")
                            nc.tensor.matmul(
                                ps_s[:, 0, mcol0:], kt_t[0:DK, p, ks],
                                qt_t[0:DK, p, mcol0:], start=True, stop=True,
                            )
                            nc.tensor.matmul(
                                ps_s[:, 1, mcol0:], kt_t[DK:P, p, ks],
                                qt_t[DK:P, p, mcol0:], start=True, stop=True,
                            )
                            if r >= 0:
                                mstart = mcol0 if r == 3 else col0
                                region = slice(mstart, col0 + P)
                                w = col0 + P - mstart
                                nc.vector.tensor_tensor(
                                    ps_s[:, :, region],
                                    ps_s[:, :, region],
                                    mask_sb[:, 2 * P - w:]
                                    .unsqueeze(1).to_broadcast([P, 2, w]),
                                    mybir.AluOpType.add,
                                )
                            e_pair = ep.tile([P, 2, 512], BF16, tag="e")
                            nc.scalar.activation(
                                e_pair[:, :, mcol0:], ps_s[:, :, mcol0:],
                                mybir.ActivationFunctionType.Exp, scale=SCALE,
                            )
                            pend.append((e_pair, mcol0, kc))
                            if len(pend) > 9:
                                emit_av(*pend.pop(0))

                    def av_part():
                        for it in st["pend"]:
                            emit_av(*it)
                        # normalize: at[f, q] = a[f, q] / a[64, q].
                        # The 1/denom row broadcast runs on GPSIMD (idle
                        # engine) instead of a PE ones-matmul.
                        for half, ps_a in ((0, st["a0"]), (1, st["a1"])):
                            inv_r = invp.tile([1, 512], F32, tag="invr")
                            with nc.allow_low_precision(reason="softmax recip"):
                                nc.vector.reciprocal(inv_r[:], ps_a[DK:DK + 1, :])
                            rep = repp.tile([DK, 512], F32, tag="rep")
                            nc.gpsimd.partition_broadcast(rep[:], inv_r[:])
                            nc.vector.tensor_tensor(
                                at_t[p][half * DK:(half + 1) * DK, :],
                                ps_a[0:DK, :], rep[:], mybir.AluOpType.mult,
                            )

                    return scores_part, av_part
                def outproj():
                    # output projection: outT[j, q] partial
                    for ob in range(OB):
                        ps_o = pss.tile([P, 512], F32, tag="s")
                        for fc in range(FC):
                            nc.tensor.matmul(
                                ps_o[:], wo_sb[:, fc, ob * P:(ob + 1) * P],
                                at_t[fc][:],
                                start=(fc == 0), stop=(fc == FC - 1),
                            )
                        o_t = outp.tile([P, 512], BF16, tag="o")
                        nc.vector.tensor_copy(o_t[:], ps_o[:])
                        nc.sync.dma_start(
                            outT_a[:, ob, qb * 512:(qb + 1) * 512], o_t[:])

                return [make_pair(p) for p in range(FC)], outproj

            # Emission. qb=0: weave the attention pairs directly into
            # phase1(0) — pair p's scores only need the fc=p chunks of
            # qt/kt, so they run as soon as (Qp, Kp) land instead of after
            # the whole projection phase. Weight streams ride the Pool
            # SWDGE queue, parallel to the wq/xt HWDGE stream, ordered to
            # match first use (wk before wv before wo). qb>=1: attention
            # (qb) interleaved with phase1(qb+1) — one block ahead of the
            # attention that consumes it.
            ph = phase1_chunks(0)       # [setup, Q0..Q3, K0..K3, V0..V3]
            ph[0]()                      # wq + xt(0) DMAs
            nc.sync.dma_start(mask_sb[:], maskb.ap())
            for dc in range(DC):
                nc.gpsimd.dma_start(wk_sb[dc][:], wkT_a[:, dc, :])
            for dc in range(DC):
                nc.gpsimd.dma_start(wv_sb[dc][:], wvT_a[:, dc, :])
            nc.gpsimd.dma_start(wo_sb[:], woT_a)
            att0, opj0 = attention_chunks(0)
            ph[1]()                      # Q0
            ph[5]()                      # K0
            ph[2]()                      # Q1 (covers K0's PSUM->SBUF copy)
            att0[0][0]()                 # scores(p0)
            ph[6]()                      # K1
            ph[3]()                      # Q2
            att0[1][0]()                 # scores(p1)
            ph[7]()                      # K2
            ph[4]()                      # Q3
            att0[2][0]()                 # scores(p2)
            ph[8]()                      # K3
            ph[9]()                      # V0 (covers K3's copy)
            att0[3][0]()                 # scores(p3)
            nxt = phase1_chunks(1)
            nxt[0]()                     # xt(1) DMA prefetch (xt double-buffered)
            ph[10]()                     # V1
            ph[11]()                     # V2
            ph[12]()                     # V3
            att0[0][1]()                 # av(p0)
            nxt[1]()                     # Q0(1) — covers the norm latency
            att0[1][1]()                 # av(p1)
            nxt[2]()                     # Q1(1)
            att0[2][1]()                 # av(p2)
            nxt[3]()                     # Q2(1)
            att0[3][1]()                 # av(p3)
            nxt[4]()                     # Q3(1)
            opjs = [opj0]
            for ch in nxt[5:]:           # K(1), V(1)
                ch()
            for qb in range(1, NB - 1):
                pairs, opj = attention_chunks(qb)
                att = [c for pr in pairs for c in pr]
                opjs.append(opj)
                nxt = phase1_chunks(qb + 1)
                seq = []
                seq.append(nxt[0])       # xt DMA prefetch first
                k = 1
                for i, pc in enumerate(att):
                    seq.append(pc)
                    n = 2 if i % 2 == 0 else 1   # sc,ph,ph,av,ph pacing
                    for _ in range(n):
                        if k < len(nxt):
                            seq.append(nxt[k])
                            k += 1
                while k < len(nxt):
                    seq.append(nxt[k])
                    k += 1
                for ch in seq:
                    ch()
            # Last block: the deferred output projections are the only
            # remaining PE-dense work — weave them between the AV parts so
            # the exp backlog on ACT is hidden behind them.
            pairs, opj3 = attention_chunks(NB - 1)
            sc, av = zip(*pairs)
            sc[0](); av[0]()
            sc[1](); opjs[0](); av[1]()
            sc[2](); opjs[1](); av[2]()
            sc[3](); opjs[2](); av[3]()
            opj3()


def _build():
    nc = bacc.Bacc("TRN2", target_bir_lowering=False, debug=False)
    _emit(nc)
    nc.compile()
    return nc


def _make_in_maps(x, W_q, W_k, W_v, W_o):
    import ml_dtypes
    bf = ml_dtypes.bfloat16
    kk = np.arange(P)[:, None]
    jj = np.arange(P)[None, :]
    band = np.where(kk <= jj, 0.0, NEG).astype(np.float32)
    # [128, 256]: first 128 cols fully masked (r=3 widened tiles), then the
    # triangular diagonal band
    maskb = np.concatenate(
        [np.full((P, P), NEG, np.float32), band], axis=1)
    in_maps = []
    for c in range(8):
        b, g = divmod(c, 2)
        cols = slice(g * FLOC, (g + 1) * FLOC)
        in_maps.append({
            "xT": np.ascontiguousarray(x[b].T).astype(bf),
            "wqT": np.ascontiguousarray(W_q[cols, :].T).astype(bf),
            "wkT": np.ascontiguousarray(W_k[cols, :].T).astype(bf),
            "wvT": np.ascontiguousarray(W_v[cols, :].T).astype(bf),
            "woT": np.ascontiguousarray(W_o[:, cols].T).astype(bf),
            "maskb": maskb,
        })
    return in_maps


def kernel(x, W_q, W_k, W_v, W_o):
    x = np.asarray(x, dtype=np.float32)
    W_q = np.asarray(W_q, dtype=np.float32)
    W_k = np.asarray(W_k, dtype=np.float32)
    W_v = np.asarray(W_v, dtype=np.float32)
    W_o = np.asarray(W_o, dtype=np.float32)
    if "nc" not in _CACHE:
        _CACHE["nc"] = _build()
    nc = _CACHE["nc"]
    in_maps = _make_in_maps(x, W_q, W_k, W_v, W_o)
    res = bass_utils.run_bass_kernel_spmd(nc, in_maps, core_ids=list(range(8)))
    B = x.shape[0]
    out = np.empty((B, S, D), dtype=np.float32)
    for b in range(B):
        acc = (res.results[2 * b]["outT"].astype(np.float32)
               + res.results[2 * b + 1]["outT"].astype(np.float32))
        out[b] = acc.T
    return out



# revision 42
# speedup vs baseline: 2.2493x; 1.0025x over previous
"""Causal multi-head attention (B=4, S=2048, D=1024, H=16, dk=64) on 8 TRN2
NeuronCores.

Sharding: core c = (batch b = c // 2, head-group g = c % 2 of 8 heads).
Each core computes, for its batch and its 8 heads:
    Q.T, K.T (feature-major) and V (seq-major) projections,
    S.T = K_h @ Q_h.T tiles (keys on partitions, queries on free dim; the
    two heads of a pair land in disjoint PE row groups and run row-tiled),
    causal mask (additive -1e30 on the 128-wide diagonal band),
    exp (scale 1/sqrt(dk) folded into the ACT activation),
    A.T = [V_h | ones].T @ expS.T accumulated in PSUM (row 64 = softmax
    denominator, obtained for free), normalization via DVE reciprocal +
    GPSIMD partition_broadcast (keeps the PE free of broadcast matmuls),
    partial out.T = W_o_slice.T-chunks @ A.T  (summed on host across the
    2 head-group cores of each batch).

All matmul operands are bf16 (fp32 PSUM accumulation; max rel err vs the
fp32 reference ~5e-3, well under the 2e-2 gate). Inputs are converted to
bf16 on the host, halving DMA traffic; output partials return as bf16 and
are summed in fp32 on the host.

Schedule: phase1(qb) projections run one query-block ahead, woven into
attention(qb-1). For qb=0 the attention pairs are woven directly into
phase1(0) per (Qp, Kp) chunk. Weight streams ride the Pool SWDGE queue in
parallel with the wq/xt HWDGE stream. All four output projections are
deferred to the final attention block, where they are the PE-dense filler
that hides the ACT exp backlog (every attention block standalone is
ACT-bound; the projections are what balance it).
"""

import numpy as np

import concourse.bacc as bacc
import concourse.tile as tile
from concourse import mybir
from concourse import bass_utils

F32 = mybir.dt.float32
F32R = mybir.dt.float32r
BF16 = mybir.dt.bfloat16
P = 128          # partitions
S = 2048         # sequence length
D = 1024         # model dim
FLOC = 512       # local features per core (8 heads x 64)
HLOC = 8         # heads per core
DK = 64
NB = 4           # seq blocks of 512 (query blocks)
KC = 16          # key chunks of 128
DC = 8           # D chunks of 128
FC = 4           # local-feature chunks of 128 (= head pairs)
OB = 8           # output-D blocks of 128
NEG = -1.0e30
SCALE = 0.125    # 1 / sqrt(dk)

_CACHE = {}


def _emit(nc):
    xT = nc.dram_tensor("xT", [D, S], BF16, kind="ExternalInput")
    wqT = nc.dram_tensor("wqT", [D, FLOC], BF16, kind="ExternalInput")
    wkT = nc.dram_tensor("wkT", [D, FLOC], BF16, kind="ExternalInput")
    wvT = nc.dram_tensor("wvT", [D, FLOC], BF16, kind="ExternalInput")
    woT = nc.dram_tensor("woT", [FLOC, D], BF16, kind="ExternalInput")
    maskb = nc.dram_tensor("maskb", [P, 2 * P], F32, kind="ExternalInput")
    outT = nc.dram_tensor("outT", [D, S], BF16, kind="ExternalOutput")

    # DRAM APs in on-chip layouts
    xT_a = xT.ap().rearrange("(c p) s -> p c s", p=P)      # [128, 8, 2048]
    wqT_a = wqT.ap().rearrange("(c p) f -> p c f", p=P)    # [128, 8, 512]
    wkT_a = wkT.ap().rearrange("(c p) f -> p c f", p=P)
    wvT_a = wvT.ap().rearrange("(c p) f -> p c f", p=P)
    woT_a = woT.ap().rearrange("(c p) j -> p c j", p=P)    # [128, 4, 1024]
    outT_a = outT.ap().rearrange("(c p) s -> p c s", p=P)                # [128, 8, 2048]

    with tile.TileContext(nc) as tc:
        import contextlib
        ctx = contextlib.ExitStack()
        with ctx:
            persist = ctx.enter_context(tc.tile_pool(name="persist", bufs=1))
            wpool = ctx.enter_context(tc.tile_pool(name="w", bufs=1))
            xtp = ctx.enter_context(tc.tile_pool(name="xt", bufs=2))
            qtp = ctx.enter_context(tc.tile_pool(name="qt", bufs=2))
            ep = ctx.enter_context(tc.tile_pool(name="e", bufs=18))
            atp = ctx.enter_context(tc.tile_pool(name="at", bufs=1))
            repp = ctx.enter_context(tc.tile_pool(name="rep", bufs=4))
            outp = ctx.enter_context(tc.tile_pool(name="out", bufs=6))
            invp = ctx.enter_context(tc.tile_pool(name="inv", bufs=4))
            pss = ctx.enter_context(tc.tile_pool(name="pss", bufs=2, space="PSUM"))
            pss2 = ctx.enter_context(tc.tile_pool(name="pss2", bufs=2, space="PSUM"))
            pav = ctx.enter_context(tc.tile_pool(name="pav", bufs=2, space="PSUM"))

            # static tiles; wq is split per D-chunk into separate tiles so
            # the first projection matmuls only wait on the pieces they read
            # (Tile tracks dependencies at whole-tile granularity).
            wq_sb = [wpool.tile([P, FLOC], BF16, tag=f"wq{dc}", name=f"wq{dc}")
                     for dc in range(DC)]
            wk_sb = [wpool.tile([P, FLOC], BF16, tag=f"wk{dc}", name=f"wk{dc}")
                     for dc in range(DC)]
            wv_sb = [wpool.tile([P, FLOC], BF16, tag=f"wv{dc}", name=f"wv{dc}")
                     for dc in range(DC)]
            wo_sb = persist.tile([P, FC, D], BF16, tag="wo")
            mask_sb = persist.tile([P, 2 * P], F32, tag="mask")

            kt_blocks = []
            v_blocks = []
            qt_blocks = []

            def phase1_chunks(sb):
                """QKV projections for seq block sb, as a list of emission
                chunks so they can be interleaved with attention work."""
                chunks = []

                state = {}

                def setup():
                    xt_t = [xtp.tile([P, 512], BF16, tag=f"xt{dc}", name=f"xt{dc}")
                            for dc in range(DC)]
                    qs = [nc.sync, nc.scalar]
                    for dc in range(DC):
                        if sb == 0:
                            # interleave the wq pieces so the first projection
                            # group streams right behind the DMA dispatches;
                            # two engine DGE queues in parallel
                            qs[dc % 2].dma_start(wq_sb[dc][:], wqT_a[:, dc, :])
                            qs[(dc + 1) % 2].dma_start(
                                xt_t[dc][:],
                                xT_a[:, dc, sb * 512:(sb + 1) * 512])
                        else:
                            nc.sync.dma_start(
                                xt_t[dc][:],
                                xT_a[:, dc, sb * 512:(sb + 1) * 512])
                    qt_t = qtp.tile([P, FC, 512], BF16, tag="qt")
                    kt_t = persist.tile([P, FC, 512], BF16, tag=f"kt{sb}")
                    v_t = persist.tile([P, 4, HLOC, DK + 1], BF16, tag=f"v{sb}")
                    qt_blocks.append(qt_t)
                    kt_blocks.append(kt_t)
                    v_blocks.append(v_t)
                    # ones column of V_aug (softmax denominators)
                    nc.gpsimd.memset(v_t[:, :, :, DK], 1.0)
                    state.update(xt=xt_t, qt=qt_t, kt=kt_t, v=v_t)

                chunks.append(setup)

                def q_chunk(fc):
                    def run():
                        ps_q = pss.tile([P, 512], F32, tag="s")
                        for dc in range(DC):
                            nc.tensor.matmul(
                                ps_q[:], wq_sb[dc][:, fc * P:(fc + 1) * P],
                                state["xt"][dc][:],
                                start=(dc == 0), stop=(dc == DC - 1),
                            )
                        nc.scalar.copy(state["qt"][:, fc, :], ps_q[:])
                    return run

                def k_chunk(fc):
                    def run():
                        ps_k = pss.tile([P, 512], F32, tag="s")
                        for dc in range(DC):
                            nc.tensor.matmul(
                                ps_k[:], wk_sb[dc][:, fc * P:(fc + 1) * P],
                                state["xt"][dc][:],
                                start=(dc == 0), stop=(dc == DC - 1),
                            )
                        nc.scalar.copy(state["kt"][:, fc, :], ps_k[:])
                    return run

                def v_chunk(sc):
                    def run():
                        ps_v = pss.tile([P, 512], F32, tag="s")
                        for dc in range(DC):
                            nc.tensor.matmul(
                                ps_v[:], state["xt"][dc][:, sc * P:(sc + 1) * P],
                                wv_sb[dc][:],
                                start=(dc == 0), stop=(dc == DC - 1),
                            )
                        nc.scalar.copy(
                            state["v"][:, sc, :, 0:DK],
                            ps_v[:].rearrange("p (h d) -> p h d", h=HLOC),
                        )
                    return run

                for fc in range(FC):
                    chunks.append(q_chunk(fc))
                for fc in range(FC):
                    chunks.append(k_chunk(fc))
                for sc in range(4):
                    chunks.append(v_chunk(sc))
                return chunks

            def phase1(sb):
                for ch in phase1_chunks(sb):
                    ch()

            def attention_chunks(qb):
                """Attention + output projection for query block qb.

                Returns (pairs, outproj) where pairs[p] = (scores_part,
                av_part): scores_part emits scores+mask+exp (and any
                pend-overflow AV), av_part flushes the remaining AV matmuls
                and normalizes. Emitting them as separate chunks lets the
                qb=0 schedule start scores before the V projections exist.
                """
                qt_t = qt_blocks[qb]
                at_t = [atp.tile([P, 512], BF16, tag=f"at{qb}_{p}",
                                 name=f"at{qb}_{p}") for p in range(FC)]
                last = 4 * qb + 3

                def make_pair(p):  # head pair
                    st = {}

                    def emit_av(e_pair, col0, kc):
                        v_t = v_blocks[kc // 4]
                        nc.tensor.matmul(
                            st["a0"][:, col0:], v_t[:, kc % 4, 2 * p, :],
                            e_pair[:, 0, col0:], start=(kc == 0), stop=(kc == last),
                        )
                        nc.tensor.matmul(
                            st["a1"][:, col0:], v_t[:, kc % 4, 2 * p + 1, :],
                            e_pair[:, 1, col0:], start=(kc == 0), stop=(kc == last),
                        )

                    def scores_part():
                        st["a0"] = pav.tile([DK + 1, 512], F32, tag="av", name="ps_a0")
                        st["a1"] = pav.tile([DK + 1, 512], F32, tag="av", name="ps_a1")
                        pend = st["pend"] = []
                        for kc in range(last + 1):
                            r = kc - 4 * qb
                            col0 = P * r if r >= 0 else 0
                            mcol0 = col0
                            kt_t = kt_blocks[kc // 4]
                            ks = slice((kc % 4) * P, (kc % 4 + 1) * P)
                            ps_s = pss2.tile([P, 2, 512], F32, tag="s2")
                            nc.tensor.matmul(
                                ps_s[:, 0, mcol0:], kt_t[0:DK, p, ks],
                                qt_t[0:DK, p, mcol0:], start=True, stop=True,
                            )
                            nc.tensor.matmul(
                                ps_s[:, 1, mcol0:], kt_t[DK:P, p, ks],
                                qt_t[DK:P, p, mcol0:], start=True, stop=True,
                            )
                            if r >= 0:
                                mstart = mcol0 if r == 3 else col0
                                region = slice(mstart, col0 + P)
                                w = col0 + P - mstart
                                nc.vector.tensor_tensor(
                                    ps_s[:, :, region],
                                    ps_s[:, :, region],
                                    mask_sb[:, 2 * P - w:]
                                    .unsqueeze(1).to_broadcast([P, 2, w]),
                                    mybir.AluOpType.add,
                                )
                            e_pair = ep.tile([P, 2, 512], BF16, tag="e")
                            nc.scalar.activation(
                                e_pair[:, :, mcol0:], ps_s[:, :, mcol0:],
                                mybir.ActivationFunctionType.Exp, scale=SCALE,
                            )
                            pend.append((e_pair, mcol0, kc))
                            if len(pend) > 9:
                                emit_av(*pend.pop(0))

                    def av_part():
                        for it in st["pend"]:
                            emit_av(*it)
                        # normalize: at[f, q] = a[f, q] / a[64, q].
                        # The 1/denom row broadcast runs on GPSIMD (idle
                        # engine) instead of a PE ones-matmul.
                        for half, ps_a in ((0, st["a0"]), (1, st["a1"])):
                            inv_r = invp.tile([1, 512], F32, tag="invr")
                            with nc.allow_low_precision(reason="softmax recip"):
                                nc.vector.reciprocal(inv_r[:], ps_a[DK:DK + 1, :])
                            rep = repp.tile([DK, 512], F32, tag="rep")
                            nc.gpsimd.partition_broadcast(rep[:], inv_r[:])
                            nc.vector.tensor_tensor(
                                at_t[p][half * DK:(half + 1) * DK, :],
                                ps_a[0:DK, :], rep[:], mybir.AluOpType.mult,
                            )

                    return scores_part, av_part
                def outproj():
                    # output projection: outT[j, q] partial; out DMAs
                    # alternate the two engine DGE queues so the tail
                    # drains at twice the single-queue pitch on hardware
                    for ob in range(OB):
                        ps_o = pss.tile([P, 512], F32, tag="s")
                        for fc in range(FC):
                            nc.tensor.matmul(
                                ps_o[:], wo_sb[:, fc, ob * P:(ob + 1) * P],
                                at_t[fc][:],
                                start=(fc == 0), stop=(fc == FC - 1),
                            )
                        o_t = outp.tile([P, 512], BF16, tag="o")
                        nc.vector.tensor_copy(o_t[:], ps_o[:])
                        oq = nc.sync if ob % 2 == 0 else nc.scalar
                        oq.dma_start(
                            outT_a[:, ob, qb * 512:(qb + 1) * 512], o_t[:])

                return [make_pair(p) for p in range(FC)], outproj

            # Emission. qb=0: weave the attention pairs directly into
            # phase1(0) — pair p's scores only need the fc=p chunks of
            # qt/kt, so they run as soon as (Qp, Kp) land instead of after
            # the whole projection phase. Weight streams ride the Pool
            # SWDGE queue, parallel to the wq/xt HWDGE stream, ordered to
            # match first use (wk before wv before wo). qb>=1: attention
            # (qb) interleaved with phase1(qb+1) — one block ahead of the
            # attention that consumes it.
            ph = phase1_chunks(0)       # [setup, Q0..Q3, K0..K3, V0..V3]
            ph[0]()                      # wq + xt(0) DMAs
            nc.sync.dma_start(mask_sb[:], maskb.ap())
            for dc in range(DC):
                nc.gpsimd.dma_start(wk_sb[dc][:], wkT_a[:, dc, :])
            for dc in range(DC):
                nc.gpsimd.dma_start(wv_sb[dc][:], wvT_a[:, dc, :])
            nc.gpsimd.dma_start(wo_sb[:], woT_a)
            att0, opj0 = attention_chunks(0)
            ph[1]()                      # Q0
            ph[5]()                      # K0
            ph[2]()                      # Q1 (covers K0's PSUM->SBUF copy)
            att0[0][0]()                 # scores(p0)
            ph[6]()                      # K1
            ph[3]()                      # Q2
            att0[1][0]()                 # scores(p1)
            ph[7]()                      # K2
            ph[4]()                      # Q3
            att0[2][0]()                 # scores(p2)
            ph[8]()                      # K3
            ph[9]()                      # V0 (covers K3's copy)
            att0[3][0]()                 # scores(p3)
            nxt = phase1_chunks(1)
            nxt[0]()                     # xt(1) DMA prefetch (xt double-buffered)
            ph[10]()                     # V1
            ph[11]()                     # V2
            ph[12]()                     # V3
            att0[0][1]()                 # av(p0)
            nxt[1]()                     # Q0(1) — covers the norm latency
            att0[1][1]()                 # av(p1)
            nxt[2]()                     # Q1(1)
            att0[2][1]()                 # av(p2)
            nxt[3]()                     # Q2(1)
            att0[3][1]()                 # av(p3)
            nxt[4]()                     # Q3(1)
            opjs = [opj0]
            for ch in nxt[5:]:           # K(1), V(1)
                ch()
            for qb in range(1, NB - 1):
                pairs, opj = attention_chunks(qb)
                att = [c for pr in pairs for c in pr]
                opjs.append(opj)
                nxt = phase1_chunks(qb + 1)
                seq = []
                seq.append(nxt[0])       # xt DMA prefetch first
                k = 1
                for i, pc in enumerate(att):
                    seq.append(pc)
                    n = 2 if i % 2 == 0 else 1   # sc,ph,ph,av,ph pacing
                    for _ in range(n):
                        if k < len(nxt):
                            seq.append(nxt[k])
                            k += 1
                while k < len(nxt):
                    seq.append(nxt[k])
                    k += 1
                for ch in seq:
                    ch()
            # Last block: the deferred output projections are the only
            # remaining PE-dense work — weave them between the AV parts so
            # the exp backlog on ACT is hidden behind them.
            pairs, opj3 = attention_chunks(NB - 1)
            sc, av = zip(*pairs)
            sc[0](); av[0]()
            sc[1](); opjs[0](); av[1]()
            sc[2](); opjs[1](); av[2]()
            sc[3](); opjs[2](); av[3]()
            opj3()


def _build():
    nc = bacc.Bacc("TRN2", target_bir_lowering=False, debug=False)
    _emit(nc)
    nc.compile()
    return nc


def _make_in_maps(x, W_q, W_k, W_v, W_o):
    import ml_dtypes
    bf = ml_dtypes.bfloat16
    kk = np.arange(P)[:, None]
    jj = np.arange(P)[None, :]
    band = np.where(kk <= jj, 0.0, NEG).astype(np.float32)
    # [128, 256]: first 128 cols fully masked (r=3 widened tiles), then the
    # triangular diagonal band
    maskb = np.concatenate(
        [np.full((P, P), NEG, np.float32), band], axis=1)
    in_maps = []
    for c in range(8):
        b, g = divmod(c, 2)
        cols = slice(g * FLOC, (g + 1) * FLOC)
        in_maps.append({
            "xT": np.ascontiguousarray(x[b].T).astype(bf),
            "wqT": np.ascontiguousarray(W_q[cols, :].T).astype(bf),
            "wkT": np.ascontiguousarray(W_k[cols, :].T).astype(bf),
            "wvT": np.ascontiguousarray(W_v[cols, :].T).astype(bf),
            "woT": np.ascontiguousarray(W_o[:, cols].T).astype(bf),
            "maskb": maskb,
        })
    return in_maps


def kernel(x, W_q, W_k, W_v, W_o):
    x = np.asarray(x, dtype=np.float32)
    W_q = np.asarray(W_q, dtype=np.float32)
    W_k = np.asarray(W_k, dtype=np.float32)
    W_v = np.asarray(W_v, dtype=np.float32)
    W_o = np.asarray(W_o, dtype=np.float32)
    if "nc" not in _CACHE:
        _CACHE["nc"] = _build()
    nc = _CACHE["nc"]
    in_maps = _make_in_maps(x, W_q, W_k, W_v, W_o)
    res = bass_utils.run_bass_kernel_spmd(nc, in_maps, core_ids=list(range(8)))
    B = x.shape[0]
    out = np.empty((B, S, D), dtype=np.float32)
    for b in range(B):
        acc = (res.results[2 * b]["outT"].astype(np.float32)
               + res.results[2 * b + 1]["outT"].astype(np.float32))
        out[b] = acc.T
    return out



# revision 43
# speedup vs baseline: 2.2558x; 1.0029x over previous
"""Causal multi-head attention (B=4, S=2048, D=1024, H=16, dk=64) on 8 TRN2
NeuronCores.

Sharding: core c = (batch b = c // 2, head-group g = c % 2 of 8 heads).
Each core computes, for its batch and its 8 heads:
    Q.T, K.T (feature-major) and V (seq-major) projections,
    S.T = K_h @ Q_h.T tiles (keys on partitions, queries on free dim; the
    two heads of a pair land in disjoint PE row groups and run row-tiled),
    causal mask (additive -1e30 on the 128-wide diagonal band),
    exp (scale 1/sqrt(dk) folded into the ACT activation),
    A.T = [V_h | ones].T @ expS.T accumulated in PSUM (row 64 = softmax
    denominator, obtained for free), normalization via DVE reciprocal +
    GPSIMD partition_broadcast (keeps the PE free of broadcast matmuls),
    partial out.T = W_o_slice.T-chunks @ A.T  (summed on host across the
    2 head-group cores of each batch).

All matmul operands are bf16 (fp32 PSUM accumulation; max rel err vs the
fp32 reference ~5e-3, well under the 2e-2 gate). Inputs are converted to
bf16 on the host, halving DMA traffic; output partials return as bf16 and
are summed in fp32 on the host.

Schedule: phase1(qb) projections run one query-block ahead, woven into
attention(qb-1). For qb=0 the attention pairs are woven directly into
phase1(0) per (Qp, Kp) chunk. Weight streams ride the Pool SWDGE queue in
parallel with the wq/xt HWDGE stream. All four output projections are
deferred to the final attention block, where they are the PE-dense filler
that hides the ACT exp backlog (every attention block standalone is
ACT-bound; the projections are what balance it).
"""

import numpy as np

import concourse.bacc as bacc
import concourse.tile as tile
from concourse import mybir
from concourse import bass_utils

F32 = mybir.dt.float32
F32R = mybir.dt.float32r
BF16 = mybir.dt.bfloat16
P = 128          # partitions
S = 2048         # sequence length
D = 1024         # model dim
FLOC = 512       # local features per core (8 heads x 64)
HLOC = 8         # heads per core
DK = 64
NB = 4           # seq blocks of 512 (query blocks)
KC = 16          # key chunks of 128
DC = 8           # D chunks of 128
FC = 4           # local-feature chunks of 128 (= head pairs)
OB = 8           # output-D blocks of 128
NEG = -1.0e30
SCALE = 0.125    # 1 / sqrt(dk)

_CACHE = {}


def _emit(nc):
    xT = nc.dram_tensor("xT", [D, S], BF16, kind="ExternalInput")
    wqT = nc.dram_tensor("wqT", [D, FLOC], BF16, kind="ExternalInput")
    wkT = nc.dram_tensor("wkT", [D, FLOC], BF16, kind="ExternalInput")
    wvT = nc.dram_tensor("wvT", [D, FLOC], BF16, kind="ExternalInput")
    woT = nc.dram_tensor("woT", [FLOC, D], BF16, kind="ExternalInput")
    maskb = nc.dram_tensor("maskb", [P, 2 * P], F32, kind="ExternalInput")
    outT = nc.dram_tensor("outT", [D, S], BF16, kind="ExternalOutput")

    # DRAM APs in on-chip layouts
    xT_a = xT.ap().rearrange("(c p) s -> p c s", p=P)      # [128, 8, 2048]
    wqT_a = wqT.ap().rearrange("(c p) f -> p c f", p=P)    # [128, 8, 512]
    wkT_a = wkT.ap().rearrange("(c p) f -> p c f", p=P)
    wvT_a = wvT.ap().rearrange("(c p) f -> p c f", p=P)
    woT_a = woT.ap().rearrange("(c p) j -> p c j", p=P)    # [128, 4, 1024]
    outT_a = outT.ap().rearrange("(c p) s -> p c s", p=P)                # [128, 8, 2048]

    with tile.TileContext(nc) as tc:
        import contextlib
        ctx = contextlib.ExitStack()
        with ctx:
            persist = ctx.enter_context(tc.tile_pool(name="persist", bufs=1))
            wpool = ctx.enter_context(tc.tile_pool(name="w", bufs=1))
            xtp = ctx.enter_context(tc.tile_pool(name="xt", bufs=2))
            qtp = ctx.enter_context(tc.tile_pool(name="qt", bufs=2))
            ep = ctx.enter_context(tc.tile_pool(name="e", bufs=18))
            atp = ctx.enter_context(tc.tile_pool(name="at", bufs=1))
            repp = ctx.enter_context(tc.tile_pool(name="rep", bufs=4))
            outp = ctx.enter_context(tc.tile_pool(name="out", bufs=6))
            invp = ctx.enter_context(tc.tile_pool(name="inv", bufs=4))
            pss = ctx.enter_context(tc.tile_pool(name="pss", bufs=2, space="PSUM"))
            pss2 = ctx.enter_context(tc.tile_pool(name="pss2", bufs=2, space="PSUM"))
            pav = ctx.enter_context(tc.tile_pool(name="pav", bufs=2, space="PSUM"))

            # static tiles; wq is split per D-chunk into separate tiles so
            # the first projection matmuls only wait on the pieces they read
            # (Tile tracks dependencies at whole-tile granularity).
            wq_sb = [wpool.tile([P, FLOC], BF16, tag=f"wq{dc}", name=f"wq{dc}")
                     for dc in range(DC)]
            wk_sb = [wpool.tile([P, FLOC], BF16, tag=f"wk{dc}", name=f"wk{dc}")
                     for dc in range(DC)]
            wv_sb = [wpool.tile([P, FLOC], BF16, tag=f"wv{dc}", name=f"wv{dc}")
                     for dc in range(DC)]
            wo_sb = persist.tile([P, FC, D], BF16, tag="wo")
            mask_sb = persist.tile([P, 2 * P], F32, tag="mask")

            kt_blocks = []
            v_blocks = []
            qt_blocks = []

            def phase1_chunks(sb):
                """QKV projections for seq block sb, as a list of emission
                chunks so they can be interleaved with attention work."""
                chunks = []

                state = {}

                def setup():
                    xt_t = [xtp.tile([P, 512], BF16, tag=f"xt{dc}", name=f"xt{dc}")
                            for dc in range(DC)]
                    qs = [nc.sync, nc.scalar]
                    for dc in range(DC):
                        if sb == 0:
                            # interleave the wq pieces so the first projection
                            # group streams right behind the DMA dispatches;
                            # two engine DGE queues in parallel
                            qs[dc % 2].dma_start(wq_sb[dc][:], wqT_a[:, dc, :])
                            qs[(dc + 1) % 2].dma_start(
                                xt_t[dc][:],
                                xT_a[:, dc, sb * 512:(sb + 1) * 512])
                        else:
                            nc.sync.dma_start(
                                xt_t[dc][:],
                                xT_a[:, dc, sb * 512:(sb + 1) * 512])
                    qt_t = qtp.tile([P, FC, 512], BF16, tag="qt")
                    kt_t = persist.tile([P, FC, 512], BF16, tag=f"kt{sb}")
                    v_t = persist.tile([P, 4, HLOC, DK + 1], BF16, tag=f"v{sb}")
                    qt_blocks.append(qt_t)
                    kt_blocks.append(kt_t)
                    v_blocks.append(v_t)
                    # ones column of V_aug (softmax denominators)
                    nc.gpsimd.memset(v_t[:, :, :, DK], 1.0)
                    state.update(xt=xt_t, qt=qt_t, kt=kt_t, v=v_t)

                chunks.append(setup)

                def q_chunk(fc):
                    def run():
                        ps_q = pss.tile([P, 512], F32, tag="s")
                        for dc in range(DC):
                            nc.tensor.matmul(
                                ps_q[:], wq_sb[dc][:, fc * P:(fc + 1) * P],
                                state["xt"][dc][:],
                                start=(dc == 0), stop=(dc == DC - 1),
                            )
                        nc.scalar.copy(state["qt"][:, fc, :], ps_q[:])
                    return run

                def k_chunk(fc):
                    def run():
                        ps_k = pss.tile([P, 512], F32, tag="s")
                        for dc in range(DC):
                            nc.tensor.matmul(
                                ps_k[:], wk_sb[dc][:, fc * P:(fc + 1) * P],
                                state["xt"][dc][:],
                                start=(dc == 0), stop=(dc == DC - 1),
                            )
                        nc.scalar.copy(state["kt"][:, fc, :], ps_k[:])
                    return run

                def v_chunk(sc):
                    def run():
                        ps_v = pss.tile([P, 512], F32, tag="s")
                        for dc in range(DC):
                            nc.tensor.matmul(
                                ps_v[:], state["xt"][dc][:, sc * P:(sc + 1) * P],
                                wv_sb[dc][:],
                                start=(dc == 0), stop=(dc == DC - 1),
                            )
                        nc.scalar.copy(
                            state["v"][:, sc, :, 0:DK],
                            ps_v[:].rearrange("p (h d) -> p h d", h=HLOC),
                        )
                    return run

                for fc in range(FC):
                    chunks.append(q_chunk(fc))
                for fc in range(FC):
                    chunks.append(k_chunk(fc))
                for sc in range(4):
                    chunks.append(v_chunk(sc))
                return chunks

            def phase1(sb):
                for ch in phase1_chunks(sb):
                    ch()

            def attention_chunks(qb):
                """Attention + output projection for query block qb.

                Returns (pairs, outproj) where pairs[p] = (scores_part,
                av_part): scores_part emits scores+mask+exp (and any
                pend-overflow AV), av_part flushes the remaining AV matmuls
                and normalizes. Emitting them as separate chunks lets the
                qb=0 schedule start scores before the V projections exist.
                """
                qt_t = qt_blocks[qb]
                at_t = [atp.tile([P, 512], BF16, tag=f"at{qb}_{p}",
                                 name=f"at{qb}_{p}") for p in range(FC)]
                last = 4 * qb + 3

                def make_pair(p):  # head pair
                    st = {}

                    def emit_av(e_pair, col0, kc):
                        v_t = v_blocks[kc // 4]
                        nc.tensor.matmul(
                            st["a0"][:, col0:], v_t[:, kc % 4, 2 * p, :],
                            e_pair[:, 0, col0:], start=(kc == 0), stop=(kc == last),
                        )
                        nc.tensor.matmul(
                            st["a1"][:, col0:], v_t[:, kc % 4, 2 * p + 1, :],
                            e_pair[:, 1, col0:], start=(kc == 0), stop=(kc == last),
                        )

                    def scores_part():
                        st["a0"] = pav.tile([DK + 1, 512], F32, tag="av", name="ps_a0")
                        st["a1"] = pav.tile([DK + 1, 512], F32, tag="av", name="ps_a1")
                        pend = st["pend"] = []
                        for kc in range(last + 1):
                            r = kc - 4 * qb
                            col0 = P * r if r >= 0 else 0
                            mcol0 = col0
                            kt_t = kt_blocks[kc // 4]
                            ks = slice((kc % 4) * P, (kc % 4 + 1) * P)
                            ps_s = pss2.tile([P, 2, 512], F32, tag="s2")
                            nc.tensor.matmul(
                                ps_s[:, 0, mcol0:], kt_t[0:DK, p, ks],
                                qt_t[0:DK, p, mcol0:], start=True, stop=True,
                            )
                            nc.tensor.matmul(
                                ps_s[:, 1, mcol0:], kt_t[DK:P, p, ks],
                                qt_t[DK:P, p, mcol0:], start=True, stop=True,
                            )
                            if r >= 0:
                                mstart = mcol0 if r == 3 else col0
                                region = slice(mstart, col0 + P)
                                w = col0 + P - mstart
                                nc.vector.tensor_tensor(
                                    ps_s[:, :, region],
                                    ps_s[:, :, region],
                                    mask_sb[:, 2 * P - w:]
                                    .unsqueeze(1).to_broadcast([P, 2, w]),
                                    mybir.AluOpType.add,
                                )
                            e_pair = ep.tile([P, 2, 512], BF16, tag="e")
                            nc.scalar.activation(
                                e_pair[:, :, mcol0:], ps_s[:, :, mcol0:],
                                mybir.ActivationFunctionType.Exp, scale=SCALE,
                            )
                            pend.append((e_pair, mcol0, kc))
                            if len(pend) > 13:
                                emit_av(*pend.pop(0))

                    def av_part():
                        for it in st["pend"]:
                            emit_av(*it)
                        # normalize: at[f, q] = a[f, q] / a[64, q].
                        # The 1/denom row broadcast runs on GPSIMD (idle
                        # engine) instead of a PE ones-matmul.
                        for half, ps_a in ((0, st["a0"]), (1, st["a1"])):
                            inv_r = invp.tile([1, 512], F32, tag="invr")
                            with nc.allow_low_precision(reason="softmax recip"):
                                nc.vector.reciprocal(inv_r[:], ps_a[DK:DK + 1, :])
                            rep = repp.tile([DK, 512], F32, tag="rep")
                            nc.gpsimd.partition_broadcast(rep[:], inv_r[:])
                            nc.vector.tensor_tensor(
                                at_t[p][half * DK:(half + 1) * DK, :],
                                ps_a[0:DK, :], rep[:], mybir.AluOpType.mult,
                            )

                    return scores_part, av_part
                def outproj():
                    # output projection: outT[j, q] partial; out DMAs
                    # alternate the two engine DGE queues so the tail
                    # drains at twice the single-queue pitch on hardware
                    for ob in range(OB):
                        ps_o = pss.tile([P, 512], F32, tag="s")
                        for fc in range(FC):
                            nc.tensor.matmul(
                                ps_o[:], wo_sb[:, fc, ob * P:(ob + 1) * P],
                                at_t[fc][:],
                                start=(fc == 0), stop=(fc == FC - 1),
                            )
                        o_t = outp.tile([P, 512], BF16, tag="o")
                        nc.vector.tensor_copy(o_t[:], ps_o[:])
                        oq = nc.sync if ob % 2 == 0 else nc.scalar
                        oq.dma_start(
                            outT_a[:, ob, qb * 512:(qb + 1) * 512], o_t[:])

                return [make_pair(p) for p in range(FC)], outproj

            # Emission. qb=0: weave the attention pairs directly into
            # phase1(0) — pair p's scores only need the fc=p chunks of
            # qt/kt, so they run as soon as (Qp, Kp) land instead of after
            # the whole projection phase. Weight streams ride the Pool
            # SWDGE queue, parallel to the wq/xt HWDGE stream, ordered to
            # match first use (wk before wv before wo). qb>=1: attention
            # (qb) interleaved with phase1(qb+1) — one block ahead of the
            # attention that consumes it.
            ph = phase1_chunks(0)       # [setup, Q0..Q3, K0..K3, V0..V3]
            ph[0]()                      # wq + xt(0) DMAs
            nc.sync.dma_start(mask_sb[:], maskb.ap())
            for dc in range(DC):
                nc.gpsimd.dma_start(wk_sb[dc][:], wkT_a[:, dc, :])
            for dc in range(DC):
                nc.gpsimd.dma_start(wv_sb[dc][:], wvT_a[:, dc, :])
            nc.gpsimd.dma_start(wo_sb[:], woT_a)
            att0, opj0 = attention_chunks(0)
            ph[1]()                      # Q0
            ph[5]()                      # K0
            ph[2]()                      # Q1 (covers K0's PSUM->SBUF copy)
            att0[0][0]()                 # scores(p0)
            ph[6]()                      # K1
            ph[3]()                      # Q2
            att0[1][0]()                 # scores(p1)
            ph[7]()                      # K2
            ph[4]()                      # Q3
            att0[2][0]()                 # scores(p2)
            ph[8]()                      # K3
            ph[9]()                      # V0 (covers K3's copy)
            att0[3][0]()                 # scores(p3)
            nxt = phase1_chunks(1)
            nxt[0]()                     # xt(1) DMA prefetch (xt double-buffered)
            ph[10]()                     # V1
            ph[11]()                     # V2
            ph[12]()                     # V3
            att0[0][1]()                 # av(p0)
            nxt[1]()                     # Q0(1) — covers the norm latency
            att0[1][1]()                 # av(p1)
            nxt[2]()                     # Q1(1)
            att0[2][1]()                 # av(p2)
            nxt[3]()                     # Q2(1)
            att0[3][1]()                 # av(p3)
            nxt[4]()                     # Q3(1)
            opjs = [opj0]
            for ch in nxt[5:]:           # K(1), V(1)
                ch()
            for qb in range(1, NB - 1):
                pairs, opj = attention_chunks(qb)
                att = [c for pr in pairs for c in pr]
                opjs.append(opj)
                nxt = phase1_chunks(qb + 1)
                seq = []
                seq.append(nxt[0])       # xt DMA prefetch first
                k = 1
                for i, pc in enumerate(att):
                    seq.append(pc)
                    n = 2 if i % 2 == 0 else 1   # sc,ph,ph,av,ph pacing
                    for _ in range(n):
                        if k < len(nxt):
                            seq.append(nxt[k])
                            k += 1
                while k < len(nxt):
                    seq.append(nxt[k])
                    k += 1
                for ch in seq:
                    ch()
            # Last block: the deferred output projections are the only
            # remaining PE-dense work — weave them between the AV parts so
            # the exp backlog on ACT is hidden behind them.
            pairs, opj3 = attention_chunks(NB - 1)
            sc, av = zip(*pairs)
            sc[0](); av[0]()
            sc[1](); opjs[0](); av[1]()
            sc[2](); opjs[1](); av[2]()
            sc[3](); opjs[2](); av[3]()
            opj3()


def _build():
    nc = bacc.Bacc("TRN2", target_bir_lowering=False, debug=False)
    _emit(nc)
    nc.compile()
    return nc


def _make_in_maps(x, W_q, W_k, W_v, W_o):
    import ml_dtypes
    bf = ml_dtypes.bfloat16
    kk = np.arange(P)[:, None]
    jj = np.arange(P)[None, :]
    band = np.where(kk <= jj, 0.0, NEG).astype(np.float32)
    # [128, 256]: first 128 cols fully masked (r=3 widened tiles), then the
    # triangular diagonal band
    maskb = np.concatenate(
        [np.full((P, P), NEG, np.float32), band], axis=1)
    in_maps = []
    for c in range(8):
        b, g = divmod(c, 2)
        cols = slice(g * FLOC, (g + 1) * FLOC)
        in_maps.append({
            "xT": np.ascontiguousarray(x[b].T).astype(bf),
            "wqT": np.ascontiguousarray(W_q[cols, :].T).astype(bf),
            "wkT": np.ascontiguousarray(W_k[cols, :].T).astype(bf),
            "wvT": np.ascontiguousarray(W_v[cols, :].T).astype(bf),
            "woT": np.ascontiguousarray(W_o[:, cols].T).astype(bf),
            "maskb": maskb,
        })
    return in_maps


def kernel(x, W_q, W_k, W_v, W_o):
    x = np.asarray(x, dtype=np.float32)
    W_q = np.asarray(W_q, dtype=np.float32)
    W_k = np.asarray(W_k, dtype=np.float32)
    W_v = np.asarray(W_v, dtype=np.float32)
    W_o = np.asarray(W_o, dtype=np.float32)
    if "nc" not in _CACHE:
        _CACHE["nc"] = _build()
    nc = _CACHE["nc"]
    in_maps = _make_in_maps(x, W_q, W_k, W_v, W_o)
    res = bass_utils.run_bass_kernel_spmd(nc, in_maps, core_ids=list(range(8)))
    B = x.shape[0]
    out = np.empty((B, S, D), dtype=np.float32)
    for b in range(B):
        acc = (res.results[2 * b]["outT"].astype(np.float32)
               + res.results[2 * b + 1]["outT"].astype(np.float32))
        out[b] = acc.T
    return out

